# revision 80
# baseline (speedup 1.0000x reference)
"""Trainium2 Bass kernel for an AttentionBlock (b=8, c=512, T=32*64=2048, 4 heads).

Data-parallel over batch: each of the 8 NeuronCores processes one batch
element end-to-end (QKV projection, attention, output projection, residual).
Weights are replicated; no collectives.

Dtype plan (gate is rel_max < 2e-2; this lands ~8e-3):
  - QKV / scores / output projection are fp8e4m3 DoubleRow matmuls
    (0.5 cycles/row: 4x fp32r for the K=512 contractions, 2x for the K=128
    score contraction via a shared zero k-tile slot addressed as a strided
    second plane).
  - PV stays bf16: exp(S) reaches e^12 which overflows fp8, and a per-row
    max is not cheaply computable in the S^T (s-on-partitions) layout.
  - exp outputs, V^T, the residual x and the DRAM output are bf16 (2x DVE
    throughput, half the HBM traffic); x also ships as fp8 for QKV.
  - v-bias is folded into the projection bias on the host (exact, since
    softmax rows sum to 1), so normalize is a single multiply.

Engine assignment (HW-validated: GPSIMD cannot touch PSUM; ScalarE is the
exp-bound engine at ~134us/iter; short matmuls pay an unmodeled LD_WEIGHTS
cost of ~230ns each on HW):
  - ScalarE: all 128 exp ops [128,1024] + half the K-projection bias-copies
    (emitted before the exp stream starts).
  - DVE: PSUM->SBUF copies (Q/K bias-add fp8, V^T bf16), odd denominator
    partial sums, reciprocal, normalize multiply, residual+bias.
  - GpSimd: even denominator partial sums (SBUF-only bf16 adds).
  - PE: scores (DoubleRow, one s-tile per weight load), PV accumulation,
    QKV/proj, denominator joins + 1/D partition-broadcast via ones-matmuls.

Scheduling: the PE FIFO is strict in-order, so emission order is execution
order.  Q/V^T projection tiles trickle in as 1-per-step PE fillers inside
the score+exp stream; PV lags the exp stream (PVLAG groups) and the
normalize chain is staggered over +1..+6 steps (joins, D copy, broadcast,
reciprocal, multiply) so every cross-engine dependency is old by the time
an engine reaches it; projection chunks follow one per step.  Each loop
iteration re-reads x and all weights from DRAM and writes the full output
back (no state is carried between iterations except compile-time zero
constants).
"""

import math

import numpy as np

import concourse.bacc as bacc
import concourse.mybir as mybir
import concourse.tile as tile
from concourse.bass_utils import run_bass_kernel_spmd

P = 128          # partitions
C = 512          # channels
T = 2048         # tokens (f*t = 32*64)
H = 4            # heads (head dim = 128 = P)
B = 8            # batch (one per core)
NC_ = C // P     # 4 c-chunks
NT = T // 512    # 4 t-blocks
NS = T // P      # 16 s-tiles
FP = mybir.dt.float32
BF = mybir.dt.bfloat16
F8 = mybir.dt.float8e4
FR = mybir.dt.float32r
DR = mybir.MatmulPerfMode.DoubleRow
EXP_GRP = 2      # s-tiles per exp() call ([128, 1024] PSUM group)
NFOLD = 8        # score groups whose denominator contribution folds on
                 # GpSimd+DVE (8 = all; join via two final ones-matmuls)

_PROGRAM_CACHE = {}


def _build_program(loop_k: int = 1, probe=None):
    from contextlib import nullcontext

    nc = bacc.Bacc()

    x8_d = nc.dram_tensor("x8", [C, T], F8, kind="ExternalInput")
    xbf_d = nc.dram_tensor("xbf", [C, T], BF, kind="ExternalInput")
    wq8_d = nc.dram_tensor("wq8", [C, C], F8, kind="ExternalInput")  # [c,o], scale folded
    wk8_d = nc.dram_tensor("wk8", [C, C], F8, kind="ExternalInput")
    wv8_d = nc.dram_tensor("wv8", [C, C], F8, kind="ExternalInput")
    pw8_d = nc.dram_tensor("pw8", [C, C], F8, kind="ExternalInput")
    bq_d = nc.dram_tensor("bq", [P, NC_], FP, kind="ExternalInput")  # [p, chunk]
    bk_d = nc.dram_tensor("bk", [P, NC_], FP, kind="ExternalInput")
    pb_d = nc.dram_tensor("pb", [P, NC_], FP, kind="ExternalInput")  # pb + Pw@bv
    z8_d = nc.dram_tensor("z8", [P, T], F8, kind="ExternalInput")    # zeros
    out_d = nc.dram_tensor("out", [C, T], BF, kind="ExternalOutput")

    x8_v = x8_d.rearrange("(j p) t -> p j t", p=P)    # [128, 4, 2048]
    xbf_v = xbf_d.rearrange("(j p) t -> p j t", p=P)
    wq_v = wq8_d.rearrange("(j p) o -> p j o", p=P)
    wk_v = wk8_d.rearrange("(j p) o -> p j o", p=P)
    wv_v = wv8_d.rearrange("(j p) o -> p j o", p=P)
    pw_v = pw8_d.rearrange("(j p) o -> p j o", p=P)
    out_v = out_d.rearrange("(j p) t -> p j t", p=P)

    with tile.TileContext(nc) as tc:
        with tc.tile_pool(name="outer", bufs=1) as outer:
            # Constants that survive across loop iterations: ones vectors and
            # the shared zero k-tile slot (index H) inside q8/k8 (written
            # once, read-only in the loop body).  The score matmuls address
            # head h's data + the zero slot as a strided 2-plane AP.
            q8_sb = [outer.tile([P, H + 1, T], F8, name=f"q8_{u}")
                     for u in range(2)]           # slot H == 0
            k8_sb = [outer.tile([P, H + 1, T], F8, name=f"k8_{u}")
                     for u in range(2)]
            ones_col = outer.tile([P, 1], BF)      # lhsT for colsum matmul
            ones_row = outer.tile([1, P], BF)      # lhsT for bcast matmul
            ones_col_f = outer.tile([P, 1], FP)
            ones_row_f = outer.tile([1, P], FP)
            nc.vector.memset(ones_col_f, 1.0)
            nc.vector.memset(ones_row_f, 1.0)
            nc.vector.tensor_copy(ones_col, ones_col_f)
            nc.vector.tensor_copy(ones_row, ones_row_f)
            # zero k-tile slots come from DRAM via the GpSimd DGE (a DVE
            # memset or an SP-queue DMA would block the K-projection path)
            for u in range(2):
                nc.gpsimd.dma_start(q8_sb[u][:, H, :], z8_d[:])
                nc.gpsimd.dma_start(k8_sb[u][:, H, :], z8_d[:])

            dram_views = (x8_v, xbf_v, wq_v, wk_v, wv_v, pw_v,
                          bq_d, bk_d, pb_d, out_v)
            with (
                tc.tile_pool(name="pp", bufs=1) as pp,
                tc.tile_pool(name="psA", bufs=2, space="PSUM") as psA,
                tc.tile_pool(name="psAcc", bufs=2, space="PSUM") as psAcc,
                tc.tile_pool(name="psS", bufs=2, space="PSUM") as psS,
                tc.tile_pool(name="epool", bufs=20) as epool,
                tc.tile_pool(name="anorm", bufs=2) as anormp,
                tc.tile_pool(name="small", bufs=2) as small,
            ):
                pools = (psA, psAcc, psS, epool, anormp, small)
                # (a 2x-unrolled ping-pong variant measured worse on HW:
                # the second body's DMAs contend with compute for SBUF
                # bandwidth, so keep one body per hardware-loop iteration)
                tiles = _alloc_body_tiles(pp, 0)
                if loop_k > 1:
                    with tc.For_i(0, loop_k):
                        _emit_kernel_body(
                            nc, tc, q8_sb[0], k8_sb[0], ones_col, ones_row,
                            *dram_views, probe=probe, tiles=tiles, uname=0,
                            pools=pools,
                        )
                else:
                    _emit_kernel_body(
                        nc, tc, q8_sb[0], k8_sb[0], ones_col, ones_row,
                        *dram_views, probe=probe, tiles=tiles, uname=0,
                        pools=pools,
                    )

    nc.compile()
    return nc


def _alloc_body_tiles(pool, u):
    return {
        "x8": pool.tile([P, NC_, T], F8, name=f"x8_{u}"),
        "xbf": pool.tile([P, NC_, T], BF, name=f"xbf_{u}"),
        "vt": pool.tile([P, NS, C], BF, name=f"vt_{u}"),
        "pw8": pool.tile([P, NC_, C], F8, name=f"pw8_{u}"),
        "bq": pool.tile([P, NC_], FP, name=f"bq_{u}"),
        "bk": pool.tile([P, NC_], FP, name=f"bk_{u}"),
        "pb": pool.tile([P, NC_], FP, name=f"pb_{u}"),
        "wq": pool.tile([P, NC_, C], F8, name=f"wq_{u}"),
        "wk": pool.tile([P, NC_, C], F8, name=f"wk_{u}"),
        "wv": pool.tile([P, NC_, C], F8, name=f"wv_{u}"),
    }


def _emit_kernel_body(nc, tc, q8_sb, k8_sb, ones_col, ones_row,
                      x8_v, xbf_v, wq_v, wk_v, wv_v, pw_v,
                      bq_d, bk_d, pb_d, out_v, probe=None,
                      tiles=None, uname=0, pools=None):
    AF = mybir.ActivationFunctionType
    # probe modes (timing-only HW experiments; outputs are garbage):
    #   'se'     : QKV + scores+exp stream (no PV/denominator/normalize/proj)
    #   'nonorm' : full kernel minus normalize/proj/out-DMA
    p_qkv = probe in (None, 'se', 'nonorm')
    p_pv = probe in (None, 'nonorm')
    p_norm = probe is None

    psA, psAcc, psS, epool, anormp, small = pools
    if True:
        x8_sb = tiles["x8"]
        xbf_sb = tiles["xbf"]
        vt_sb = tiles["vt"]
        pw8_sb = tiles["pw8"]
        bq_sb = tiles["bq"]
        bk_sb = tiles["bk"]
        pb_sb = tiles["pb"]
        wq_sb = tiles["wq"]
        wk_sb = tiles["wk"]
        wv_sb = tiles["wv"]

        if p_qkv:
            nc.sync.dma_start(bq_sb, bq_d[:])
            nc.sync.dma_start(bk_sb, bk_d[:])

        # ---- phase A DMAs: K's dependencies first so exp starts early.
        # Few, large transfers: each dma_start costs ~0.7us of SP sequencing,
        # so per-chunk triggers would gate the whole pipeline start.
        if p_qkv:
            nc.sync.dma_start(wk_sb, wk_v)
            nc.sync.dma_start(x8_sb[:, :, 0:512], x8_v[:, :, 0:512])
            nc.sync.dma_start(x8_sb[:, :, 512:1024], x8_v[:, :, 512:1024])
            nc.sync.dma_start(wq_sb, wq_v)
            nc.sync.dma_start(x8_sb[:, :, 1024:1536], x8_v[:, :, 1024:1536])
            nc.sync.dma_start(x8_sb[:, :, 1536:2048], x8_v[:, :, 1536:2048])
            nc.sync.dma_start(wv_sb, wv_v)
            nc.sync.dma_start(pw8_sb, pw_v)
            nc.sync.dma_start(pb_sb, pb_d[:])
            nc.sync.dma_start(xbf_sb[:, :, 0:1024], xbf_v[:, :, 0:1024])
            nc.sync.dma_start(xbf_sb[:, :, 1024:2048], xbf_v[:, :, 1024:2048])

        # Q/K projection tile: out[o_tile, t] = sum_g W8[c_g, ot].T @ x8[c_g, t]
        # (DoubleRow: two 128-channel k-tiles per matmul)
        def emit_qk_tile(w_sb, b_sb, dst, tb, ot, copy_eng="vector",
                         pool=None):
            if pool is None:
                ps = psS.tile([P, 512], FP, tag="s", name=f"kq{tb}_{ot}_{uname}")
            else:
                ps = pool.tile([P, 1024], FP, tag="mm",
                               name=f"kq{tb}_{ot}_{uname}")[:, 0:512]
            for g in range(2):
                nc.tensor.matmul(
                    ps,
                    w_sb[:, 2 * g:2 * g + 2, ot * P:(ot + 1) * P],
                    x8_sb[:, 2 * g:2 * g + 2, tb * 512:(tb + 1) * 512],
                    start=(g == 0),
                    stop=(g == 1),
                    perf_mode=DR,
                )
            dst_ap = dst[:, ot, tb * 512:(tb + 1) * 512]
            # bias-add + fp8 store (GPSIMD cannot read PSUM; ScalarE helps
            # only where it would otherwise idle, e.g. the K projection)
            if copy_eng == "act":
                nc.scalar.activation(dst_ap, ps,
                                     mybir.ActivationFunctionType.Identity,
                                     bias=b_sb[:, ot:ot + 1])
            else:
                nc.vector.tensor_scalar_add(dst_ap, ps, b_sb[:, ot:ot + 1])

        # V^T tile: out[s_tile, o] = sum_g x8[c_g, s_tile].T @ Wv8[c_g, o]
        def emit_vt_tile(st, pool=None):
            if pool is None:
                ps = psS.tile([P, 512], FP, tag="s", name=f"vt{st}_{uname}")
            else:
                ps = pool.tile([P, 1024], FP, tag="mm",
                               name=f"vt{st}_{uname}")[:, 0:512]
            for g in range(2):
                nc.tensor.matmul(
                    ps,
                    x8_sb[:, 2 * g:2 * g + 2, st * P:(st + 1) * P],
                    wv_sb[:, 2 * g:2 * g + 2, :],
                    start=(g == 0),
                    stop=(g == 1),
                    perf_mode=DR,
                )
            nc.vector.tensor_copy(vt_sb[:, st, :], ps)

        # K first (scores for any t-block read all of k8), then the first Q
        # tile; the remaining Q/V^T tiles are interleaved into the score
        # stream as PE fillers.
        fillers = []
        if p_qkv:
            # alternate psA/psS so four K tiles are in flight (two copy
            # chains, ACT + DVE, truly overlapped) during the startup window
            for i, (tb, ot) in enumerate([(tb, ot) for tb in range(NT)
                                          for ot in range(NC_)]):
                emit_qk_tile(wk_sb, bk_sb, k8_sb, tb, ot,
                             copy_eng="act" if i % 2 else "vector",
                             pool=psA if i % 2 else None)
            emit_qk_tile(wq_sb, bq_sb, q8_sb, 0, 0, pool=psA)

            for ot in range(1, NC_):
                fillers.append(("q", 0, ot))
            for st in range(NS):
                fillers.append(("vt", st))
            for tb in range(1, NT):
                for ot in range(NC_):
                    fillers.append(("q", tb, ot))

        def pop_fillers(step):
            if not fillers:
                return
            f = fillers.pop(0)
            if f[0] == "q":
                emit_qk_tile(wq_sb, bq_sb, q8_sb, f[1], f[2])
            else:
                emit_vt_tile(f[1])

        # ---- phase B/C: attention + projection, software-pipelined ----
        # The PE engine queue is strict FIFO, so emission order == PE
        # execution order.  PV/denominator work lags the S^T matmul + exp
        # stream by PVLAG groups; normalize runs as soon as an iteration's
        # last PV pops (the next iteration's PV would deadlock behind it in
        # the PE FIFO otherwise); projection chunks are staggered.
        if True:
            PVLAG = 12          # while PE fillers pending
            PVLAG_STEADY = 4    # once fillers drained

            NGR = NS // EXP_GRP                    # 8 groups per (h, tb)
            iters = [(h, tb) for tb in range(NT) for h in range(H)]
            NIT = len(iters)

            acc = {}   # it -> [a_ps, d_ps, fp_sb, fv_sb]
            an = {}    # tb -> an_sb tile

            def emit_pv(it, g, e_sb):
                h, tb = iters[it]
                if g == 0:
                    acc[it] = [
                        psAcc.tile([P, 512], FP, tag="acc", name=f"aps{it}_{uname}"),
                        None,   # d_ps, allocated at the joins event
                        None,   # GpSimd-fold partial
                        None,   # DVE-fold partial
                    ]
                a_ps, _d, fp_sb, fv_sb = acc[it]
                for u in range(EXP_GRP):
                    st = g * EXP_GRP + u
                    nc.tensor.matmul(
                        a_ps,
                        vt_sb[:, st, h * P:(h + 1) * P],
                        e_sb[:, u * 512:(u + 1) * 512],
                        start=(st == 0),
                        stop=(st == NS - 1),
                    )
                # denominator via SBUF-only folding: two independent partial
                # sums (GpSimd + DVE) so neither chain crosses engines; both
                # partials join d_ps via ones-matmuls (the 'joins' event).
                if g == 0:
                    fp_sb = small.tile([P, 512], BF, tag="foldp",
                                       name=f"foldp{it}_{uname}")
                    fv_sb = small.tile([P, 512], BF, tag="foldv",
                                       name=f"foldv{it}_{uname}")
                    acc[it][2] = fp_sb
                    acc[it][3] = fv_sb
                    nc.gpsimd.tensor_copy(fp_sb, e_sb[:, 0:512])
                    nc.vector.tensor_copy(fv_sb, e_sb[:, 512:1024])
                else:
                    nc.gpsimd.tensor_add(fp_sb, fp_sb, e_sb[:, 0:512])
                    nc.vector.tensor_add(fv_sb, fv_sb, e_sb[:, 512:1024])

            # --- staggered post-PV chain: each stage's cross-engine
            # dependency is >=1 step old when the consuming engine reaches
            # it, so exposed semaphore waits do not head-of-line-block the
            # PE/DVE FIFOs (the HW's sem latency is far larger than the
            # cost model's). ---
            def ev_joins(it):
                d_ps = psS.tile([1, 512], FP, tag="s", name=f"dps{it}_{uname}")
                acc[it][1] = d_ps
                _, _, fp_sb, fv_sb = acc[it]
                nc.tensor.matmul(d_ps, ones_col, fp_sb, start=True, stop=False)
                nc.tensor.matmul(d_ps, ones_col, fv_sb, start=False, stop=True)

            def ev_dcopy(it):
                d_sb = small.tile([1, 512], BF, tag="dsb", name=f"dsb{it}_{uname}")
                acc[it].append(d_sb)
                with nc.allow_low_precision(reason="bf16 broadcast of D; a "
                                            "0.4% row-uniform scale error is "
                                            "inside the error budget"):
                    nc.vector.tensor_copy(d_sb, acc[it][1])

            def ev_bcast(it):
                b_ps = psS.tile([P, 512], FP, tag="s", name=f"bps{it}_{uname}")
                acc[it].append(b_ps)
                nc.tensor.matmul(b_ps, ones_row, acc[it][4],
                                 start=True, stop=True)

            def ev_recip(it):
                r_sb = small.tile([P, 512], FP, tag="rsb", name=f"rsb{it}_{uname}")
                acc[it].append(r_sb)
                nc.vector.reciprocal(r_sb, acc[it][5])

            def ev_mul(it):
                h, tb = iters[it]
                if h == 0:
                    an[tb] = anormp.tile([P, NC_, 512], F8, tag="anorm",
                                         name=f"an{tb}_{uname}")
                a_ps = acc[it][0]
                r_sb = acc[it][6]
                nc.vector.tensor_mul(an[tb][:, h, :], a_ps, r_sb)
                acc.pop(it)

            def ev_proj(pl):
                tb, ot = pl
                tsl = slice(tb * 512, (tb + 1) * 512)
                an_sb = an[tb]
                hp = psS.tile([P, 512], FP, tag="s", name=f"hp{tb}_{ot}_{uname}")
                for g in range(2):
                    nc.tensor.matmul(
                        hp,
                        pw8_sb[:, 2 * g:2 * g + 2, ot * P:(ot + 1) * P],
                        an_sb[:, 2 * g:2 * g + 2, :],
                        start=(g == 0),
                        stop=(g == 1),
                        perf_mode=DR,
                    )
                o_sb = small.tile([P, 512], BF, tag="osb", bufs=3)
                # out = (hp + pb') + x  in one DVE op
                nc.vector.scalar_tensor_tensor(
                    o_sb, hp, pb_sb[:, ot:ot + 1], xbf_sb[:, ot, tsl],
                    op0=mybir.AluOpType.add, op1=mybir.AluOpType.add,
                )
                nc.sync.dma_start(out_v[:, ot, tsl], o_sb)

            EV = {"joins": ev_joins, "dcopy": ev_dcopy, "bcast": ev_bcast,
                  "recip": ev_recip, "mul": ev_mul, "proj": ev_proj}
            events = []   # sorted (due_step, seq, kind, payload)
            ev_seq = [0]

            def push(due, kind, payload):
                import bisect
                item = (due, ev_seq[0], kind, payload)
                ev_seq[0] += 1
                bisect.insort(events, item)

            def flush(step):
                while events and events[0][0] <= step:
                    _, _, kind, pl = events.pop(0)
                    EV[kind](pl)

            def pop_pv(step):
                pit, pg, pe_sb = pv_q.pop(0)
                emit_pv(pit, pg, pe_sb)
                if pg == NGR - 1 and p_norm:
                    push(step + 1, "joins", pit)
                    push(step + 2, "dcopy", pit)
                    push(step + 4, "bcast", pit)
                    push(step + 5, "recip", pit)
                    push(step + 6, "mul", pit)
                    nh, ntb = iters[pit]
                    if nh == H - 1:
                        for k in range(NC_):
                            push(step + 7 + k, "proj", (ntb, k))
                elif pg == NGR - 1:
                    acc.pop(pit)

            flat = [(it, g) for it in range(NIT) for g in range(NGR)]
            pv_q = []             # queue of (it, g, e_sb)
            for step, (it, g) in enumerate(flat):
                h, tb = iters[it]
                tsl = slice(tb * 512, (tb + 1) * 512)
                s_ps = psA.tile([P, 512 * EXP_GRP], FP, tag="mm",
                                name=f"sps{it}_{g}_{uname}")
                for u in range(EXP_GRP):
                    st = g * EXP_GRP + u
                    nc.tensor.matmul(
                        s_ps[:, u * 512:(u + 1) * 512],
                        k8_sb[:, h:H + 1:(H - h), st * P:(st + 1) * P],
                        q8_sb[:, h:H + 1:(H - h), tsl],
                        start=True,
                        stop=True,
                        perf_mode=DR,
                    )
                e_sb = epool.tile([P, 512 * EXP_GRP], BF, tag="e",
                                  name=f"e{it}_{g}_{uname}")
                nc.scalar.activation(e_sb, s_ps, AF.Exp)
                pop_fillers(step)
                if p_pv:
                    pv_q.append((it, g, e_sb))
                    lag = PVLAG if fillers else PVLAG_STEADY
                    while len(pv_q) > lag:
                        pop_pv(step)
                flush(step)

            # drain the pipeline tail, one virtual step at a time
            step = len(flat)
            while pv_q or events:
                if pv_q:
                    pop_pv(step)
                flush(step)
                step += 1


def _prepare_in_maps(x, qkv_w, qkv_b, proj_w, proj_b):
    import ml_dtypes

    scale = 1.0 / math.sqrt(math.sqrt(C // H))
    x = np.ascontiguousarray(np.asarray(x, dtype=np.float32).reshape(B, C, T))
    qkv_w = np.asarray(qkv_w, dtype=np.float32)
    qkv_b = np.asarray(qkv_b, dtype=np.float32)
    proj_w = np.asarray(proj_w, dtype=np.float32)
    proj_b = np.asarray(proj_b, dtype=np.float32)

    e4 = ml_dtypes.float8_e4m3
    bf = ml_dtypes.bfloat16
    wq8 = np.ascontiguousarray((qkv_w[0:C] * scale).T.astype(e4))      # [c, o]
    wk8 = np.ascontiguousarray((qkv_w[C:2 * C] * scale).T.astype(e4))
    wv8 = np.ascontiguousarray(qkv_w[2 * C:3 * C].T.astype(e4))
    pw8 = np.ascontiguousarray(proj_w.T.astype(e4))
    bq = np.ascontiguousarray((qkv_b[0:C] * scale).reshape(NC_, P).T)  # [p, chunk]
    bk = np.ascontiguousarray((qkv_b[C:2 * C] * scale).reshape(NC_, P).T)
    # v-bias folded through the projection (exact: softmax rows sum to 1)
    pb2 = proj_w @ qkv_b[2 * C:3 * C] + proj_b
    pb = np.ascontiguousarray(pb2.reshape(NC_, P).T)

    shared = {
        "wq8": wq8, "wk8": wk8, "wv8": wv8, "pw8": pw8,
        "bq": bq, "bk": bk, "pb": pb,
        "z8": np.zeros((P, T), dtype=e4),
    }
    return [
        {
            "x8": np.ascontiguousarray(x[i].astype(e4)),
            "xbf": np.ascontiguousarray(x[i].astype(bf)),
            **shared,
        }
        for i in range(B)
    ]


def run(inputs, trace=False, **spmd_kwargs):
    """Run the kernel; returns (output [8,512,32,64], BassKernelResults)."""
    if "nc" not in _PROGRAM_CACHE:
        _PROGRAM_CACHE["nc"] = _build_program()
    nc = _PROGRAM_CACHE["nc"]
    in_maps = _prepare_in_maps(
        inputs["x"], inputs["qkv_w"], inputs["qkv_b"],
        inputs["proj_w"], inputs["proj_b"],
    )
    res = run_bass_kernel_spmd(nc, in_maps, list(range(B)), trace=trace, **spmd_kwargs)
    out = np.stack(
        [np.asarray(res.results[i]["out"]).astype(np.float32) for i in range(B)]
    )
    f = 32
    return out.reshape(B, C, f, T // f), res


def kernel(x, qkv_w, qkv_b, proj_w, proj_b):
    out, _ = run(
        {"x": x, "qkv_w": qkv_w, "qkv_b": qkv_b, "proj_w": proj_w, "proj_b": proj_b}
    )
    return out
